# revision 31
# baseline (speedup 1.0000x reference)
"""Grouped MoE MLP (64 experts) on 8 Trainium2 NeuronCores.

Strategy: expert parallelism. Each core owns 8 experts (size-sorted "snake"
assignment so every core gets the same per-slot padded token capacity and the
padding is tight). Host pre-lays-out every tensor so device DMAs are large
fully-contiguous transfers. Per slot, x/w1/w2 are fused into one DRAM blob:

    xw[s]  : [128, KO*Cmax | FO*KO*128 | FO*H]  (x | w1 fo-major | w2 fo-major)
             slots 1-7: two DMAs per slot, (x|w1) then (w2), on the sync ring;
             slot 0: (x|w1-fo0) then ramped w1 fo-chunks + 2x2MB w2 chunks so
             mm1 starts as soon as the first chunk lands
    outT[s]: [128 oi, OO*Cmax] (oi, oo, t)  one ~0.6 MB bf16 DMA per slot on
             the scalar HWDGE ring (keeps outputs out of the weight FIFO)

A 36-matmul dummy warmup runs during the initial DMA wait so the PE HAM
clock-gate is already at 2.4 GHz when real matmuls arrive.

Both matmuls keep weights stationary and stream tokens as the moving operand:

    hT[f, t]   = w1t[e] (stationary, [h,f] tiles) @ xT (moving, [h, t])
    hT         = gelu(hT)                     (ScalarE, PSUM f32 -> SBUF bf16)
    outT[o, t] = w2[e] (stationary, [f,o] tiles) @ hT (moving, [f, t])

Weights stream HBM->SBUF once per core (double-buffered), PSUM accumulates in
f32, output is written back bf16 and upcast + un-permuted on host.
"""

import numpy as np

NCORES = 8
SLOTS = 8  # experts per core
NE = 64
H = 1024
F = 2048
T = 16384
P = 128
KO = H // P  # 8  k-tiles for mm1 (contraction over H)
FO = F // P  # 16 f-tiles (mm1 output tiles / mm2 contraction)
OO = H // P  # 8  output h-tiles for mm2
NMAX = 512  # max moving-operand length (one fp32 PSUM bank)

ACT_FN = "Gelu"  # overridable for CoreSim tests (Gelu not implemented there)

_prog_cache = {}


def _build_program(C):
    """Build the SPMD Bass program for per-slot token capacities C (len SLOTS)."""
    from contextlib import ExitStack

    import concourse.tile as tile
    from concourse import bacc, mybir
    from concourse.bass import MemorySpace

    bf16 = mybir.dt.bfloat16
    f32 = mybir.dt.float32
    Cmax = int(max(C))
    CmaxB = min(Cmax, NMAX)  # chunked tile width

    XOFF = 0
    W1OFF = KO * Cmax
    W2OFF = W1OFF + FO * KO * P
    XWLEN = W2OFF + FO * H

    nc = bacc.Bacc("TRN2", target_bir_lowering=False, debug=False, num_devices=NCORES)
    xw_d = nc.dram_tensor("xw", [SLOTS, P, XWLEN], bf16, kind="ExternalInput").ap()
    outT_d = nc.dram_tensor(
        "outT", [SLOTS, P, OO * Cmax], bf16, kind="ExternalOutput"
    ).ap()

    def w1s(xw_sb, fo, k):  # stationary [128 hi, 128 f'] tile for (fo, k)
        base = W1OFF + fo * KO * P + k * P
        return xw_sb[:, base : base + P]

    def w2s(w2_sb, fo, oo):  # stationary [128 fi, 128 h'] tile for (fo, oo)
        base = fo * H + oo * P
        return w2_sb[:, base : base + P]

    def xs(xw_sb, k, nb, NB):  # moving [128 hi, NB] tokens for k-tile
        base = XOFF + k * Cmax + nb
        return xw_sb[:, base : base + NB]

    with tile.TileContext(nc) as tc, ExitStack() as ctx:
        xw1_pool = ctx.enter_context(tc.tile_pool(name="xw1", bufs=2))
        w2_pool = ctx.enter_context(tc.tile_pool(name="w2", bufs=2))
        h_pool = ctx.enter_context(tc.tile_pool(name="h", bufs=2))
        o_pool = ctx.enter_context(tc.tile_pool(name="o", bufs=2))
        ph_pool = ctx.enter_context(
            tc.tile_pool(name="ph", bufs=3, space=MemorySpace.PSUM)
        )
        po_pool = ctx.enter_context(
            tc.tile_pool(name="po", bufs=3, space=MemorySpace.PSUM)
        )

        # HAM warmup: dummy matmuls during the initial DMA wait so the PE
        # clock gate is already at 8/8 (2.4 GHz) when real matmuls arrive
        warm_sb = h_pool.tile([P, P], bf16, tag="warm", name="warm_sb")
        warm_ps = ph_pool.tile([P, NMAX], f32, tag="ph", name="warm_ps")
        nc.vector.memset(warm_sb, 0)
        for _ in range(36):
            nc.tensor.matmul(warm_ps[:, :P], warm_sb, warm_sb, start=True, stop=True)

        for j in range(SLOTS):
            Cj = int(C[j])
            xw_sb = xw1_pool.tile([P, W2OFF], bf16, tag="xw1")
            w2_sb = w2_pool.tile([P, FO * H], bf16, tag="w2")
            if j == 0:
                # split so mm1/mm2 start as soon as their first chunks land;
                # chunks sized >=0.5 MB to keep DMA descriptors efficient
                cb = W1OFF + KO * P  # x fused with first fo-group of w1
                nc.sync.dma_start(xw_sb[:, :cb], xw_d[j, :, :cb])
                for nfo in (1, 2, 4, 4, 4):  # fo-groups per further w1 chunk
                    ch = nfo * KO * P
                    nc.sync.dma_start(xw_sb[:, cb : cb + ch], xw_d[j, :, cb : cb + ch])
                    cb += ch
                w2ch = 8 * H  # 8 fo-groups = 2 MB
                for cb in range(0, FO * H, w2ch):
                    nc.sync.dma_start(
                        w2_sb[:, cb : cb + w2ch], xw_d[j, :, W2OFF + cb : W2OFF + cb + w2ch]
                    )
            else:
                # three pieces: (x|w1 fo0-7) unblocks mm1's first half as soon
                # as it lands (jitter robustness), (w1 fo8-15), then w2
                mid = W1OFF + (FO // 2) * KO * P
                nc.sync.dma_start(xw_sb[:, :mid], xw_d[j, :, :mid])
                nc.sync.dma_start(xw_sb[:, mid:], xw_d[j, :, mid:W2OFF])
                nc.sync.dma_start(w2_sb, xw_d[j, :, W2OFF:])
            o_sb = o_pool.tile([P, OO * Cmax], bf16, tag="o")

            for nb in range(0, Cj, NMAX):
                NB = min(NMAX, Cj - nb)
                h_sb = h_pool.tile([P, FO * CmaxB], bf16, tag="h")
                for fo in range(FO):
                    ph = ph_pool.tile([P, NMAX], f32, tag="ph")
                    for k in range(KO):
                        nc.tensor.matmul(
                            ph[:, :NB],
                            w1s(xw_sb, fo, k),
                            xs(xw_sb, k, nb, NB),
                            start=(k == 0),
                            stop=(k == KO - 1),
                        )
                    nc.scalar.activation(
                        h_sb[:, fo * CmaxB : fo * CmaxB + NB],
                        ph[:, :NB],
                        getattr(mybir.ActivationFunctionType, ACT_FN),
                    )
                for oo in range(OO):
                    po = po_pool.tile([P, NMAX], f32, tag="po")
                    for fo in range(FO):
                        nc.tensor.matmul(
                            po[:, :NB],
                            w2s(w2_sb, fo, oo),
                            h_sb[:, fo * CmaxB : fo * CmaxB + NB],
                            start=(fo == 0),
                            stop=(fo == FO - 1),
                        )
                    nc.vector.tensor_copy(
                        o_sb[:, oo * Cmax + nb : oo * Cmax + nb + NB], po[:, :NB]
                    )
                    if j == SLOTS - 1 and nb + NB >= Cj:
                        # stream the final slot's output per-oo to shorten the tail
                        nc.scalar.dma_start(
                            outT_d[j, :, oo * Cmax : (oo + 1) * Cmax],
                            o_sb[:, oo * Cmax : (oo + 1) * Cmax],
                        )
            if j != SLOTS - 1:
                nc.scalar.dma_start(outT_d[j], o_sb)

    nc.compile()
    return nc


def _get_program(C):
    key = tuple(int(c) for c in C)
    if key not in _prog_cache:
        _prog_cache[key] = _build_program(key)
    return _prog_cache[key]


def plan(sizes):
    """Expert->core/slot assignment + slot capacities from token counts."""
    sizes = np.asarray(sizes, np.int64)
    assert sizes.shape == (NE,) and sizes.sum() == T
    order = np.argsort(-sizes, kind="stable")  # descending
    # expert_of[core][slot]
    expert_of = [[int(order[s * NCORES + c]) for s in range(SLOTS)] for c in range(NCORES)]
    C = []
    for s in range(SLOTS):
        m = max(int(sizes[order[s * NCORES + c]]) for c in range(NCORES))
        C.append(max(16, -(-m // 8) * 8))  # round up to multiple of 8, min 16
    return expert_of, C


def prepare_inputs(x, w1, w2, sizes, expert_of, C):
    """Host-side shard/pad/transpose/cast. Returns per-core input maps."""
    import ml_dtypes

    bf16 = ml_dtypes.bfloat16
    x = np.asarray(x, np.float32)
    tok_offs = np.concatenate([[0], np.cumsum(sizes)]).astype(np.int64)
    w1_bf = np.asarray(w1, np.float32).astype(bf16)  # [NE, F, H]
    w2_bf = np.asarray(w2, np.float32).astype(bf16)  # [NE, F, H]
    Cmax = int(max(C))
    W1OFF = KO * Cmax
    W2OFF = W1OFF + FO * KO * P
    XWLEN = W2OFF + FO * H

    in_maps = []
    for c in range(NCORES):
        experts = expert_of[c]
        xw = np.zeros((SLOTS, P, XWLEN), bf16)
        # w1t: [s, hi, fo, k, f']; stationary tile (fo,k) = w1[e][fo*128+f', k*128+hi].T
        xw[:, :, W1OFF:W2OFF] = (
            w1_bf[experts]
            .reshape(SLOTS, FO, P, KO, P)  # [s, fo, f', k, hi]
            .transpose(0, 4, 1, 3, 2)  # [s, hi, fo, k, f']
            .reshape(SLOTS, P, FO * KO * P)
        )
        # w2: [s, fi, fo, h]
        xw[:, :, W2OFF:] = (
            w2_bf[experts].reshape(SLOTS, FO, P, H).transpose(0, 2, 1, 3)
            .reshape(SLOTS, P, FO * H)
        )
        # xT: [s, hi, k, t]
        for s, e in enumerate(experts):
            n = int(sizes[e])
            xe = x[tok_offs[e] : tok_offs[e] + n]  # [n, H]
            xT = np.zeros((P, KO, Cmax), np.float32)
            xT[:, :, :n] = xe.T.reshape(KO, P, n).transpose(1, 0, 2)
            xw[s, :, :W1OFF] = xT.reshape(P, KO * Cmax)
        in_maps.append({"xw": xw})
    return in_maps


def scatter_output(results, sizes, expert_of, C):
    """Gather per-core transposed outputs back into the full [T, H] f32 output."""
    tok_offs = np.concatenate([[0], np.cumsum(sizes)]).astype(np.int64)
    Cmax = int(max(C))
    out = np.empty((T, H), np.float32)
    for c in range(NCORES):
        # [S, oi, oo, t] -> per expert [H, n] -> [n, H]
        outT_c = np.asarray(results[c]["outT"]).reshape(SLOTS, P, OO, Cmax)
        for s, e in enumerate(expert_of[c]):
            n = int(sizes[e])
            blk = outT_c[s, :, :, :n].astype(np.float32)  # [oi, oo, n]
            out[tok_offs[e] : tok_offs[e] + n] = (
                blk.transpose(1, 0, 2).reshape(H, n).T
            )
    return out


LAST_RUN = None  # BassKernelResults from the most recent kernel() call


def kernel(x, w1, w2, tokens_per_expert):
    global LAST_RUN
    from concourse import bass_utils

    sizes = np.asarray(tokens_per_expert, np.int64)
    expert_of, C = plan(sizes)
    nc = _get_program(C)
    in_maps = prepare_inputs(x, w1, w2, sizes, expert_of, C)
    res = bass_utils.run_bass_kernel_spmd(nc, in_maps, core_ids=list(range(NCORES)))
    LAST_RUN = res
    return scatter_output(res.results, sizes, expert_of, C)
